# revision 1
# baseline (speedup 1.0000x reference)
"""DisentangledGNN Trainium2 kernel (8 NeuronCores, SPMD).

Strategy: target-bucketed node sharding. Each core owns n/8 consecutive
nodes and every edge whose target lands in that range. Per core:
  P0: pca matmul + leaky_relu + grouped l2norm for its own node slice
  P1: AllGather of the normalized features (bf16, padded to 192 cols)
  P2: one-time gather z = H[src] for its edges (indirect DMA)
  P3: 3 routing iterations, chunk-fused: for each 128-node chunk, edges
      are processed as 128-edge tiles; per-edge "gather u[trg]" and the
      segment-sum are one-hot matmuls on the tensor engine (S matrices
      built once per chunk via iota/is_equal); softmax over the 10
      factors is exp/sum on free axis (|s|<=1 so no max subtraction).
  P4: (fused in last iteration) leaky_relu + classifier matmul.
No inter-core communication during routing: a chunk's new u depends
only on that chunk's old u and the iteration-fixed z.
"""

import numpy as np
import ml_dtypes

import concourse.bass as bass
import concourse.mybir as mybir
import concourse.tile as tile
from concourse.masks import make_identity
from concourse.bass_utils import run_bass_kernel_spmd

F32 = mybir.dt.float32
BF16 = mybir.dt.bfloat16
I32 = mybir.dt.int32
I16 = mybir.dt.int16

K = 10
SLOPE = 0.01
NITER = 3
P = 128


def _split_multiwaits(nc):
    # This walrus accepts at most 1 sync wait per instruction (2 for
    # EventSemaphore ops); split extras onto preceding same-engine NOPs.
    n = [0]
    for fn in nc.m.functions:
        for blk in fn.blocks:
            newinsts = []
            changed = False
            for ins in blk.instructions:
                si = ins.sync_info
                cap = 2 if "EventSem" in type(ins).__name__ else 1
                if si is not None and len(si.on_wait) > cap:
                    waits = list(si.on_wait)
                    for w in waits[cap:]:
                        n[0] += 1
                        nop = mybir.InstNoOp(name=f"{ins.name}-ws{n[0]}", ins=[], outs=[])
                        nop.engine = ins.engine
                        nop.sync_info = mybir.SyncInfo(on_wait=[w], on_update=[])
                        newinsts.append(nop)
                    si.on_wait = waits[:cap]
                    ins.sync_info = si
                    changed = True
                newinsts.append(ins)
            if changed:
                blk.instructions = newinsts


def _host_prep(x, edge_index, n_cores):
    """Bucket edges by target core, chunk them by 128-node blocks,
    equalize per-chunk tile counts across cores, build per-core arrays."""
    n = x.shape[0]
    npc = n // n_cores            # nodes per core
    nchunks = (npc + P - 1) // P  # 128-node chunks per core
    src = np.asarray(edge_index[0], np.int64)
    trg = np.asarray(edge_index[1], np.int64)

    core_of = trg // npc
    ltrg = trg - core_of * npc

    # sort edges by (core, ltrg) once
    order = np.lexsort((ltrg, core_of))
    src_s, ltrg_s, core_s = src[order], ltrg[order], core_of[order]

    # per (core, chunk) counts
    chunk_s = ltrg_s // P
    counts = np.zeros((n_cores, nchunks), np.int64)
    np.add.at(counts, (core_s, chunk_s), 1)
    nt = np.maximum(1, (counts.max(axis=0) + P - 1) // P)  # tiles per chunk (shared)
    T = int(nt.sum())

    src_arr = np.zeros((n_cores, T * P), np.int32)
    lloc_arr = np.full((n_cores, T * P), 255, np.int16)  # 255 = dummy, never matches
    tile_of_chunk = np.concatenate([[0], np.cumsum(nt)]).astype(np.int64)

    core_starts = np.searchsorted(core_s, np.arange(n_cores + 1))
    for c in range(n_cores):
        cs, ce = core_starts[c], core_starts[c + 1]
        chunk_c = chunk_s[cs:ce]
        starts = np.searchsorted(chunk_c, np.arange(nchunks + 1))
        for j in range(nchunks):
            e0, e1 = cs + starts[j], cs + starts[j + 1]
            base = int(tile_of_chunk[j]) * P
            cnt = e1 - e0
            src_arr[c, base:base + cnt] = src_s[e0:e1]
            lloc_arr[c, base:base + cnt] = (ltrg_s[e0:e1] - j * P).astype(np.int16)

    # device wants [128, T] partition-major: edge t*128+p -> [p, t]
    src_dev = src_arr.reshape(n_cores, T, P).transpose(0, 2, 1).copy()
    lloc_dev = lloc_arr.reshape(n_cores, T, P).transpose(0, 2, 1).copy()
    return nt, T, src_dev, lloc_dev, npc, nchunks


def build_program(nfeat, d, nclass, npc, nchunks, nt, T, n_cores, n_nodes):
    dd = d // K
    kf = nfeat  # contraction for pca, padded to x128 on host
    kf_pad = ((nfeat + P - 1) // P) * P
    npc_pad = nchunks * P
    DPAD = d

    nc = bass.Bass(num_devices=n_cores)

    xT_t = nc.dram_tensor("xT", [kf_pad, npc_pad], F32, kind="ExternalInput")
    w_t = nc.dram_tensor("pca_w", [kf_pad, d], F32, kind="ExternalInput")
    brep_t = nc.dram_tensor("pca_b_rep", [P, d], F32, kind="ExternalInput")
    cw_t = nc.dram_tensor("clf_w", [d, nclass], F32, kind="ExternalInput")
    cbrep_t = nc.dram_tensor("clf_b_rep", [P, nclass], F32, kind="ExternalInput")
    src_t = nc.dram_tensor("src", [P, T], I32, kind="ExternalInput")
    lloc_t = nc.dram_tensor("lloc", [P, T], I16, kind="ExternalInput")
    y_t = nc.dram_tensor("y", [npc_pad, nclass], F32, kind="ExternalOutput")
    Hp = nc.dram_tensor("Hp", [n_nodes, DPAD], BF16, kind="Internal")

    with tile.TileContext(nc) as tc:
        with (
            tc.tile_pool(name="persist", bufs=1) as pp,
            tc.tile_pool(name="dram", bufs=1, space="DRAM") as dp,
            tc.tile_pool(name="sb", bufs=3) as sb,
            tc.tile_pool(name="sb1", bufs=2) as sb1,
            tc.tile_pool(name="schunk", bufs=2) as sc,
            tc.tile_pool(name="ps", bufs=3, space="PSUM") as psp,
            tc.tile_pool(name="pst", bufs=1, space="PSUM") as pst,
            tc.tile_pool(name="psu", bufs=1, space="PSUM") as psu,
        ):
            # ---------------- constants / persistent state ----------------
            iota_rep = pp.tile([P, P], I16)
            nc.gpsimd.iota(iota_rep[:], pattern=[[1, P]], base=0, channel_multiplier=0)
            ident = pp.tile([P, P], BF16)
            make_identity(nc, ident[:])
            idf = pp.tile([P, P], F32)
            make_identity(nc, idf[:])

            nkt0 = kf_pad // P
            w_sb = pp.tile([P, nkt0 * d], F32)  # pca_w K-tiles side by side
            nc.sync.dma_start(
                out=w_sb[:].rearrange("p (a d) -> p a d", d=d),
                in_=w_t[:].rearrange("(a p) d -> p a d", p=P),
            )
            brep = pp.tile([P, d], F32)
            nc.sync.dma_start(out=brep[:], in_=brep_t[:])
            cw_sb = pp.tile([P, 2 * nclass], F32)  # clf_w K-tiles: [0:128], [128:160]
            nc.sync.dma_start(out=cw_sb[:, :nclass], in_=cw_t[:P, :])
            nc.sync.dma_start(out=cw_sb[: d - P, nclass:], in_=cw_t[P:, :])
            cbrep = pp.tile([P, nclass], F32)
            nc.sync.dma_start(out=cbrep[:], in_=cbrep_t[:])

            hn = pp.tile([P, nchunks * d], F32)   # normalized features, own nodes
            nc.vector.memset(hn[:], 0.0)
            eps_b = pp.tile([P, 1], F32)
            nc.vector.memset(eps_b[:], 1e-24)

            # bounce buffers for allgather
            ag_in = dp.tile([npc, DPAD], BF16)

            # ---------------- P0: pca + lrelu + l2norm --------------------
            nkt = kf_pad // P
            for m in range(nchunks):
                xt = sb.tile([P, nkt * P], F32, tag="xt")
                nc.sync.dma_start(
                    out=xt[:].rearrange("p (a q) -> p a q", q=P),
                    in_=xT_t[:, m * P:(m + 1) * P].rearrange("(a p) q -> p a q", p=P),
                )
                h_ps = psp.tile([P, d], F32, space="PSUM", tag="big")
                for a in range(nkt):
                    nc.tensor.matmul(
                        out=h_ps[:],
                        lhsT=xt[:, a * P:(a + 1) * P],
                        rhs=w_sb[:, a * d:(a + 1) * d],
                        start=(a == 0),
                        stop=(a == nkt - 1),
                    )
                h = sb.tile([P, d], F32, tag="h_sb")
                nc.vector.tensor_add(out=h[:], in0=h_ps[:], in1=brep[:])
                hs = sb.tile([P, d], F32, tag="hs")
                nc.vector.tensor_scalar_mul(out=hs[:], in0=h[:], scalar1=SLOPE)
                nc.vector.tensor_tensor(out=h[:], in0=h[:], in1=hs[:], op=mybir.AluOpType.max)
                # grouped l2 norm
                sq = sb.tile([P, d], F32, tag="sq")
                nc.vector.tensor_mul(out=sq[:], in0=h[:], in1=h[:])
                ss = sb.tile([P, K], F32, tag="ss")
                nc.vector.reduce_sum(
                    out=ss[:], in_=sq[:].rearrange("p (k e) -> p k e", k=K),
                    axis=mybir.AxisListType.X,
                )
                nrm = sb.tile([P, K], F32, tag="nrm")
                nc.scalar.activation(out=nrm[:], in_=ss[:], func=mybir.ActivationFunctionType.Sqrt)
                nc.vector.tensor_scalar_max(out=nrm[:], in0=nrm[:], scalar1=1e-12)
                rr = sb.tile([P, K], F32, tag="rr")
                nc.vector.reciprocal(out=rr[:], in_=nrm[:])
                nc.vector.tensor_tensor(
                    out=hn[:, m * d:(m + 1) * d].rearrange("p (k e) -> p k e", k=K),
                    in0=h[:].rearrange("p (k e) -> p k e", k=K),
                    in1=rr[:].unsqueeze(2).to_broadcast([P, K, dd]),
                    op=mybir.AluOpType.mult,
                )
                # bf16 padded copy for allgather
                hb = sb.tile([P, DPAD], BF16, tag="hb")
                nc.vector.tensor_copy(out=hb[:], in_=hn[:, m * d:(m + 1) * d])
                rows = min(P, npc - m * P)
                nc.sync.dma_start(out=ag_in[m * P:m * P + rows, :], in_=hb[:rows, :])

            # ---------------- P1: allgather -------------------------------
            nc.gpsimd.collective_compute(
                "AllGather",
                mybir.AluOpType.bypass,
                replica_groups=[list(range(n_cores))],
                ins=[ag_in[:]],
                outs=[Hp.ap()],
            )

            # ---------------- P2: (z gathered per chunk in P3) -------------
            src_sb = pp.tile([P, T], I32)
            nc.sync.dma_start(out=src_sb[:], in_=src_t[:])

            # ---------------- P3: routing ---------------------------------
            lloc_all = pp.tile([P, T], I16)
            nc.sync.dma_start(out=lloc_all[:], in_=lloc_t[:])

            GT = 8  # tiles per DVE batch group

            def chunk_prologue(j):
                t0, t1 = int(np.sum(nt[:j])), int(np.sum(nt[:j + 1]))
                ntj = t1 - t0
                zch = sc.tile([P, ntj * d], BF16, tag=f"zch{j % 3}")
                for t in range(ntj):
                    nc.gpsimd.indirect_dma_start(
                        out=zch[:, t * d:(t + 1) * d],
                        out_offset=None,
                        in_=Hp.ap(),
                        in_offset=bass.IndirectOffsetOnAxis(
                            ap=src_sb[:, t0 + t:t0 + t + 1], axis=0
                        ),
                    )
                S_sb = sc.tile([P, ntj * P], BF16, tag=f"S{j % 3}")
                ST_sb = sc.tile([P, ntj * P], BF16, tag=f"ST{j % 3}")
                for t in range(ntj):
                    nc.vector.tensor_tensor(
                        out=S_sb[:, t * P:(t + 1) * P],
                        in0=lloc_all[:, t0 + t:t0 + t + 1].to_broadcast([P, P]),
                        in1=iota_rep[:],
                        op=mybir.AluOpType.is_equal,
                    )
                for b0 in range(0, ntj, 4):
                    bn = min(4, ntj - b0)
                    tr_ps = pst.tile([P, 4 * P], BF16, space="PSUM", tag="tr")
                    for t in range(bn):
                        nc.tensor.transpose(
                            out=tr_ps[:, t * P:(t + 1) * P],
                            in_=S_sb[:, (b0 + t) * P:(b0 + t + 1) * P],
                            identity=ident[:],
                        )
                    nc.scalar.copy(
                        out=ST_sb[:, b0 * P:(b0 + bn) * P], in_=tr_ps[:, :bn * P]
                    )
                u_j = sc.tile([P, d], BF16, tag=f"uj{j % 3}")
                nc.vector.tensor_copy(out=u_j[:], in_=hn[:, j * d:(j + 1) * d])
                return dict(j=j, ntj=ntj, zch=zch, S_sb=S_sb, ST_sb=ST_sb, u_j=u_j)

            def chunk_groups(st):
                j, ntj, zch, S_sb, ST_sb, u_j = (
                    st["j"], st["ntj"], st["zch"], st["S_sb"], st["ST_sb"], st["u_j"]
                )
                seg_ps = psu.tile([P, d], F32, space="PSUM", tag=f"seg{j % 3}")
                st["seg_ps"] = seg_ps
                for g0 in range(0, ntj, GT):
                    gn = min(GT, ntj - g0)
                    ut_bf = sb1.tile([P, GT * d], BF16, tag="utbf")
                    for b0 in range(g0, g0 + gn, 3):
                        bn = min(3, g0 + gn - b0)
                        ut_ps = psp.tile([P, 3 * d], F32, space="PSUM", tag="big")
                        for t in range(bn):
                            nc.tensor.matmul(
                                out=ut_ps[:, t * d:(t + 1) * d],
                                lhsT=ST_sb[:, (b0 + t) * P:(b0 + t + 1) * P],
                                rhs=u_j[:],
                                start=True, stop=True,
                            )
                        nc.scalar.copy(
                            out=ut_bf[:, (b0 - g0) * d:(b0 - g0 + bn) * d],
                            in_=ut_ps[:, :bn * d],
                        )
                    prod = sb1.tile([P, GT * d], BF16, tag="prod")
                    nc.vector.tensor_mul(
                        out=prod[:, :gn * d],
                        in0=zch[:, g0 * d:(g0 + gn) * d],
                        in1=ut_bf[:, :gn * d],
                    )
                    t1_ = sb1.tile([P, GT * d // 2], BF16, tag="t1")
                    nc.vector.tensor_add(
                        out=t1_[:, :gn * d // 2].rearrange("p (a e) -> p a e", e=8),
                        in0=prod[:, :gn * d].rearrange("p (a e) -> p a e", e=dd)[:, :, 0:8],
                        in1=prod[:, :gn * d].rearrange("p (a e) -> p a e", e=dd)[:, :, 8:16],
                    )
                    t2_ = sb1.tile([P, GT * d // 4], BF16, tag="t2")
                    nc.vector.tensor_add(
                        out=t2_[:, :gn * d // 4].rearrange("p (a e) -> p a e", e=4),
                        in0=t1_[:, :gn * d // 2].rearrange("p (a e) -> p a e", e=8)[:, :, 0:4],
                        in1=t1_[:, :gn * d // 2].rearrange("p (a e) -> p a e", e=8)[:, :, 4:8],
                    )
                    t3_ = sb1.tile([P, GT * d // 8], BF16, tag="t3")
                    nc.vector.tensor_add(
                        out=t3_[:, :gn * d // 8].rearrange("p (a e) -> p a e", e=2),
                        in0=t2_[:, :gn * d // 4].rearrange("p (a e) -> p a e", e=4)[:, :, 0:2],
                        in1=t2_[:, :gn * d // 4].rearrange("p (a e) -> p a e", e=4)[:, :, 2:4],
                    )
                    s_f = sb1.tile([P, GT * K], F32, tag="sf")
                    nc.vector.tensor_add(
                        out=s_f[:, :gn * K],
                        in0=t3_[:, :gn * d // 8].rearrange("p (a e) -> p a e", e=2)[:, :, 0:1].squeeze(2),
                        in1=t3_[:, :gn * d // 8].rearrange("p (a e) -> p a e", e=2)[:, :, 1:2].squeeze(2),
                    )
                    e_f = sb1.tile([P, GT * K], F32, tag="ef")
                    nc.scalar.activation(
                        out=e_f[:, :gn * K], in_=s_f[:, :gn * K],
                        func=mybir.ActivationFunctionType.Exp,
                    )
                    q_f = sb1.tile([P, GT], F32, tag="qf")
                    nc.vector.reduce_sum(
                        out=q_f[:, :gn],
                        in_=e_f[:, :gn * K].rearrange("p (a k) -> p a k", k=K),
                        axis=mybir.AxisListType.X,
                    )
                    r_f = sb1.tile([P, GT], F32, tag="rf")
                    nc.vector.reciprocal(out=r_f[:, :gn], in_=q_f[:, :gn])
                    pe_f = sb1.tile([P, GT * K], BF16, tag="pef")
                    nc.vector.tensor_tensor(
                        out=pe_f[:, :gn * K].rearrange("p (a k) -> p a k", k=K),
                        in0=e_f[:, :gn * K].rearrange("p (a k) -> p a k", k=K),
                        in1=r_f[:, :gn].unsqueeze(2).to_broadcast([P, gn, K]),
                        op=mybir.AluOpType.mult,
                    )
                    pex = sb1.tile([P, GT * d], BF16, tag="pex")
                    nc.scalar.activation(
                        out=pex[:, :gn * d].rearrange("p (a e) -> p a e", e=dd),
                        in_=pe_f[:, :gn * K].unsqueeze(2).to_broadcast([P, gn * K, dd]),
                        func=mybir.ActivationFunctionType.Copy,
                    )
                    msg = sb1.tile([P, GT * d], BF16, tag="msg")
                    nc.vector.tensor_mul(
                        out=msg[:, :gn * d],
                        in0=zch[:, g0 * d:(g0 + gn) * d],
                        in1=pex[:, :gn * d],
                    )
                    for t in range(gn):
                        nc.tensor.matmul(
                            out=seg_ps[:],
                            lhsT=S_sb[:, (g0 + t) * P:(g0 + t + 1) * P],
                            rhs=msg[:, t * d:(t + 1) * d],
                            start=(g0 + t == 0),
                            stop=(g0 + t == ntj - 1),
                        )

            def chunk_epilogue(st, it):
                j, u_j, seg_ps = st["j"], st["u_j"], st["seg_ps"]
                tt = sc.tile([P, d], F32, tag="tt")
                nc.vector.tensor_add(out=tt[:], in0=seg_ps[:], in1=hn[:, j * d:(j + 1) * d])
                sq2 = sc.tile([P, d], F32, tag="sq2")
                nc.vector.tensor_mul(out=sq2[:], in0=tt[:], in1=tt[:])
                ss2 = sc.tile([P, K], F32, tag="ss2")
                nc.vector.reduce_sum(
                    out=ss2[:], in_=sq2[:].rearrange("p (k e) -> p k e", k=K),
                    axis=mybir.AxisListType.X,
                )
                nr2 = sc.tile([P, K], F32, tag="nr2")
                nc.scalar.activation(
                    out=nr2[:], in_=ss2[:], func=mybir.ActivationFunctionType.Sqrt,
                    bias=eps_b[:, :1],
                )
                rr2 = sc.tile([P, K], F32, tag="rr2")
                nc.vector.reciprocal(out=rr2[:], in_=nr2[:])
                if it < NITER - 1:
                    nc.vector.tensor_tensor(
                        out=u_j[:].rearrange("p (k e) -> p k e", k=K),
                        in0=tt[:].rearrange("p (k e) -> p k e", k=K),
                        in1=rr2[:].unsqueeze(2).to_broadcast([P, K, dd]),
                        op=mybir.AluOpType.mult,
                    )
                else:
                    uf = sc.tile([P, d], F32, tag="uf")
                    nc.vector.tensor_tensor(
                        out=uf[:].rearrange("p (k e) -> p k e", k=K),
                        in0=tt[:].rearrange("p (k e) -> p k e", k=K),
                        in1=rr2[:].unsqueeze(2).to_broadcast([P, K, dd]),
                        op=mybir.AluOpType.mult,
                    )
                    us = sc.tile([P, d], F32, tag="us")
                    nc.vector.tensor_scalar_mul(out=us[:], in0=uf[:], scalar1=SLOPE)
                    nc.vector.tensor_tensor(out=uf[:], in0=uf[:], in1=us[:], op=mybir.AluOpType.max)
                    uT_ps = psp.tile([P, 2 * P], F32, space="PSUM", tag="big")
                    nc.tensor.transpose(out=uT_ps[:, :P], in_=uf[:, :P], identity=idf[:])
                    nc.tensor.transpose(
                        out=uT_ps[: d - P, P:2 * P], in_=uf[:, P:d], identity=idf[:]
                    )
                    uT = sc.tile([P, 2 * P], F32, tag="uTs")
                    nc.vector.tensor_copy(out=uT[:, :P], in_=uT_ps[:, :P])
                    nc.vector.tensor_copy(out=uT[: d - P, P:], in_=uT_ps[: d - P, P:])
                    y_ps = psp.tile([P, nclass], F32, space="PSUM", tag="big")
                    nc.tensor.matmul(
                        out=y_ps[:], lhsT=uT[:, :P], rhs=cw_sb[:, :nclass],
                        start=True, stop=False,
                    )
                    nc.tensor.matmul(
                        out=y_ps[:], lhsT=uT[: d - P, P:], rhs=cw_sb[: d - P, nclass:],
                        start=False, stop=True,
                    )
                    y_sb = sc.tile([P, nclass], F32, tag="ysb")
                    nc.vector.tensor_add(out=y_sb[:], in0=y_ps[:], in1=cbrep[:])
                    nc.sync.dma_start(out=y_t[j * P:(j + 1) * P, :], in_=y_sb[:])

            # interleave chunk triples so each chunk's epilogue chain hides
            # under the other chunks' bulk work
            for j0 in range(0, nchunks, 3):
                sts = [chunk_prologue(j) for j in range(j0, min(j0 + 3, nchunks))]
                for it in range(NITER):
                    for st in sts:
                        chunk_groups(st)
                    for st in sts:
                        chunk_epilogue(st, it)
    return nc


_CACHE = {}
TRACE = False
LAST_RESULTS = None


def kernel(x, edge_index, pca_w, pca_b, clf_w, clf_b, n_cores=8, _sim=False):
    x = np.asarray(x, np.float32)
    edge_index = np.asarray(edge_index)
    idx_dtype = edge_index.dtype
    pca_w = np.asarray(pca_w, np.float32)
    pca_b = np.asarray(pca_b, np.float32)
    clf_w = np.asarray(clf_w, np.float32)
    clf_b = np.asarray(clf_b, np.float32)

    n, nfeat = x.shape
    d = pca_w.shape[1]
    nclass = clf_w.shape[1]

    nt, T, src_dev, lloc_dev, npc, nchunks = _host_prep(x, edge_index, n_cores)

    key = (n, nfeat, d, nclass, tuple(nt.tolist()))
    if key not in _CACHE:
        _CACHE[key] = build_program(nfeat, d, nclass, npc, nchunks, nt, T, n_cores, n)
        if not _sim:
            _split_multiwaits(_CACHE[key])
    nc = _CACHE[key]

    kf_pad = ((nfeat + P - 1) // P) * P
    npc_pad = nchunks * P
    w_pad = np.zeros((kf_pad, d), np.float32)
    w_pad[:nfeat] = pca_w
    brep = np.broadcast_to(pca_b, (P, d)).copy()
    cbrep = np.broadcast_to(clf_b, (P, nclass)).copy()

    in_maps = []
    for c in range(n_cores):
        xc = x[c * npc:(c + 1) * npc]
        xT = np.zeros((kf_pad, npc_pad), np.float32)
        xT[:nfeat, :npc] = xc.T
        in_maps.append({
            "xT": xT,
            "pca_w": w_pad,
            "pca_b_rep": brep,
            "clf_w": clf_w,
            "clf_b_rep": cbrep,
            "src": src_dev[c],
            "lloc": lloc_dev[c],
        })

    if _sim:
        from concourse.bass_interp import CoreSim
        assert n_cores == 1
        sim = CoreSim(nc)
        for kk, vv in in_maps[0].items():
            sim.tensor(kk)[:] = vv
        sim.simulate()
        return np.asarray(sim.tensor("y"))[:npc].astype(np.float32)
    global LAST_RESULTS
    res = run_bass_kernel_spmd(
        nc, in_maps, core_ids=list(range(n_cores)), trace=TRACE
    )
    LAST_RESULTS = res
    y = np.concatenate([res.results[c]["y"][:npc] for c in range(n_cores)], axis=0)
    return y.astype(np.float32)


if __name__ == "__main__":
    import pickle, time
    with open("/tmp/ref_inputs.pkl", "rb") as f:
        inputs = pickle.load(f)
    t0 = time.time()
    y = kernel(**inputs)
    print("kernel() wall time", time.time() - t0)
    np.save("/tmp/kernel_out.npy", y)



# revision 6
# speedup vs baseline: 1.3194x; 1.3194x over previous
"""DisentangledGNN Trainium2 kernel (8 NeuronCores, SPMD) — v2.

Strategy: target-bucketed node sharding (each core owns n/8 nodes and all
edges targeting them), with a host-side degree-balanced node permutation so
every (core, chunk) bucket holds ~equal edge counts.

Per core:
  P0  pca matmul (bf16, bias via ones-row) + leaky_relu + grouped l2norm
      (1/sqrt via exp(-0.5*ln(x)) so the Act engine never switches
      activation tables away from the exp/ln set)
  P1  AllGather of normalized features, split into 4 sub-collectives
      overlapped under P0
  P2  z = Hp[src] edge gather (indirect DMA, batched 4 tiles/instruction)
  P3  3 routing iterations; per 128-edge tile the u[trg] gather and the
      segment-sum scatter are one-hot matmuls whose fp8 mask matrices are
      precomputed on host and streamed via DMA.  Softmax over the 10
      factors: exp on Act, sums/reciprocal on DVE, and the p-broadcast to
      dd=16 via a bf16-pair trick (each p duplicated into a bf16 pair,
      bitcast f32, broadcast x8 on Act = half the elements).
  P4  (last iteration) leaky_relu + classifier matmul, bias via ones-row.
No inter-core communication during routing.
"""

import numpy as np
import ml_dtypes

import concourse.bass as bass
import concourse.mybir as mybir
import concourse.tile as tile
from concourse.masks import make_identity
from concourse.bass_utils import run_bass_kernel_spmd

F32 = mybir.dt.float32
BF16 = mybir.dt.bfloat16
I32 = mybir.dt.int32
FP8 = mybir.dt.float8e4
AF = mybir.ActivationFunctionType
AX = mybir.AxisListType
OP = mybir.AluOpType

K = 10
SLOPE = 0.01
NITER = 3
P = 128
ZBATCH = 1   # tiles per indirect-DMA gather (HW SWDGE only honors [P,1] offsets)
ZBUFS = 12   # chunks of z kept in SBUF (prefetch window)


def _split_multiwaits(nc):
    # This walrus accepts at most 1 sync wait per instruction (2 for
    # EventSemaphore ops); split extras onto preceding same-engine NOPs.
    n = [0]
    for fn in nc.m.functions:
        for blk in fn.blocks:
            newinsts = []
            changed = False
            for ins in blk.instructions:
                si = ins.sync_info
                cap = 2 if "EventSem" in type(ins).__name__ else 1
                if si is not None and len(si.on_wait) > cap:
                    waits = list(si.on_wait)
                    for w in waits[cap:]:
                        n[0] += 1
                        nop = mybir.InstNoOp(name=f"{ins.name}-ws{n[0]}", ins=[], outs=[])
                        nop.engine = ins.engine
                        nop.sync_info = mybir.SyncInfo(on_wait=[w], on_update=[])
                        newinsts.append(nop)
                    si.on_wait = waits[:cap]
                    ins.sync_info = si
                    changed = True
                newinsts.append(ins)
            if changed:
                blk.instructions = newinsts


def _host_prep(x, edge_index, n_cores):
    """Degree-balanced node->(core,chunk,slot) assignment, edge bucketing,
    fp8 one-hot mask matrices, permuted bf16 xT, Hp row mapping."""
    n, nfeat = x.shape
    npc = n // n_cores
    nchunks = (npc + P - 1) // P
    npc_pad = nchunks * P
    src = np.asarray(edge_index[0], np.int64)
    trg = np.asarray(edge_index[1], np.int64)

    deg = np.bincount(trg, minlength=n).astype(np.int64)

    # Greedy: nodes in descending-degree order to the (core,chunk) bin with
    # the fewest edges, subject to <=128 nodes/bin and npc nodes/core.
    order = np.argsort(-deg, kind="stable")
    bin_edges = np.zeros((n_cores, nchunks), np.int64)
    bin_nodes = np.zeros((n_cores, nchunks), np.int64)
    core_nodes = np.zeros(n_cores, np.int64)
    node_core = np.empty(n, np.int32)
    node_chunk = np.empty(n, np.int32)
    node_slot = np.empty(n, np.int32)
    INF = 1 << 60
    for nd in order:
        feas = (bin_nodes < P) & (core_nodes[:, None] < npc)
        masked = np.where(feas, bin_edges, INF)
        ci = int(np.argmin(masked))
        c, j = divmod(ci, nchunks)
        node_core[nd] = c
        node_chunk[nd] = j
        node_slot[nd] = bin_nodes[c, j]
        bin_nodes[c, j] += 1
        core_nodes[c] += 1
        bin_edges[c, j] += deg[nd]

    nt = np.maximum(1, (bin_edges.max(axis=0) + P - 1) // P).astype(np.int64)
    T = int(nt.sum())
    tile_of_chunk = np.concatenate([[0], np.cumsum(nt)]).astype(np.int64)

    # AllGather split points (chunk granularity) and Hp row mapping.
    nsplit = min(4, nchunks)
    bounds = [round(q * nchunks / nsplit) for q in range(nsplit + 1)]
    rows_q = [(bounds[q + 1] - bounds[q]) * P for q in range(nsplit)]
    hq_base = np.concatenate([[0], np.cumsum([n_cores * r for r in rows_q])])
    pos_in_core = node_chunk * P + node_slot
    node_split = np.searchsorted(np.asarray(bounds[1:]) * P, pos_in_core, side="right")
    hp_row = (
        hq_base[node_split]
        + node_core * np.asarray(rows_q)[node_split]
        + (pos_in_core - np.asarray(bounds)[node_split] * P)
    ).astype(np.int32)

    # Edge bucketing per core, chunk-sorted; slots padded with lloc=255.
    e_core = node_core[trg]
    e_chunk = node_chunk[trg]
    e_lloc = node_slot[trg]
    e_srow = hp_row[src]
    eorder = np.lexsort((e_lloc, e_chunk, e_core))
    e_core, e_chunk, e_lloc, e_srow = (
        e_core[eorder], e_chunk[eorder], e_lloc[eorder], e_srow[eorder])

    src_arr = np.zeros((n_cores, T * P), np.int32)
    lloc_arr = np.full((n_cores, T * P), 255, np.int32)
    core_starts = np.searchsorted(e_core, np.arange(n_cores + 1))
    for c in range(n_cores):
        cs, ce = core_starts[c], core_starts[c + 1]
        chunk_c = e_chunk[cs:ce]
        starts = np.searchsorted(chunk_c, np.arange(nchunks + 1))
        for j in range(nchunks):
            e0, e1 = cs + starts[j], cs + starts[j + 1]
            base = int(tile_of_chunk[j]) * P
            cnt = e1 - e0
            src_arr[c, base:base + cnt] = e_srow[e0:e1]
            lloc_arr[c, base:base + cnt] = e_lloc[e0:e1]

    # Device layouts: slot s -> tile s//P, lane s%P  => [P, T]
    src_dev = src_arr.reshape(n_cores, T, P).transpose(0, 2, 1).copy()
    lloc_mat = lloc_arr.reshape(n_cores, T, P).transpose(0, 2, 1)  # [c, P, T]

    # fp8 one-hot masks.  S[e-lane, t, v] = (lloc==v); ST is per-tile transpose.
    ar = np.arange(P)
    S_bool = lloc_mat[:, :, :, None] == ar[None, None, None, :]     # [c,P,T,128]
    ST_bool = S_bool.transpose(0, 3, 2, 1)                          # [c,P,T,128]
    S_dev = S_bool.astype(ml_dtypes.float8_e4m3fn).reshape(n_cores, P, T * P)
    ST_dev = np.ascontiguousarray(ST_bool).astype(ml_dtypes.float8_e4m3fn).reshape(n_cores, P, T * P)

    # Permuted xT in bf16, ones row for the pca bias.
    kf_pad = ((nfeat + 1 + P - 1) // P) * P
    xT = np.zeros((n_cores, kf_pad, npc_pad), ml_dtypes.bfloat16)
    xb = x.astype(ml_dtypes.bfloat16)
    for c in range(n_cores):
        nodes_c = np.where(node_core == c)[0]
        xT[c][:nfeat, pos_in_core[nodes_c]] = xb[nodes_c].T
    xT[:, nfeat, :] = 1.0

    meta = dict(npc=npc, nchunks=nchunks, npc_pad=npc_pad, nt=nt, T=T,
                tile_of_chunk=tile_of_chunk, bounds=bounds, rows_q=rows_q,
                hq_base=hq_base, kf_pad=kf_pad,
                node_core=node_core, pos_in_core=pos_in_core)
    return meta, src_dev, S_dev, ST_dev, xT


def _group_plan(ntj):
    """Split a chunk's ntj tiles into vector groups over the 2-bank ut
    supertile.  Returns list of (g0, gn, spans, copy_views) where spans are
    F32-element offsets into the [P,1024] supertile and copy_views describe
    (bank0_tiles, bank1_tiles) for the Act copy."""
    plan = []
    g0 = 0
    while g0 < ntj:
        gn = min(6, ntj - g0)
        if gn == 6:
            spans = [0, 160, 320, 512, 672, 832]
            views = (3, 3)
        elif gn == 5:
            spans = [0, 160, 320, 512, 672]
            views = (3, 2)
        elif gn == 4:
            spans = [0, 160, 512, 672]
            views = (2, 2)
        else:
            spans = [160 * i for i in range(gn)]
            views = (gn, 0)
        plan.append((g0, gn, spans, views))
        g0 += gn
    return plan


def build_program(nfeat, d, nclass, meta, n_cores):
    dd = d // K
    npc_pad = meta["npc_pad"]
    nchunks = meta["nchunks"]
    nt = meta["nt"]
    T = meta["T"]
    toc = meta["tile_of_chunk"]
    bounds = meta["bounds"]
    rows_q = meta["rows_q"]
    hq_base = meta["hq_base"]
    kf_pad = meta["kf_pad"]
    nkt = kf_pad // P
    HROWS = int(hq_base[-1])
    max_nt = int(nt.max())

    nc = bass.Bass(num_devices=n_cores)

    xT_t = nc.dram_tensor("xT", [kf_pad, npc_pad], BF16, kind="ExternalInput")
    w_t = nc.dram_tensor("wp", [kf_pad, d], BF16, kind="ExternalInput")
    cw_t = nc.dram_tensor("cwp", [P, 3 * nclass], BF16, kind="ExternalInput")
    src_t = nc.dram_tensor("src", [P, T], I32, kind="ExternalInput")
    S_t = nc.dram_tensor("Smask", [P, T * P], FP8, kind="ExternalInput")
    ST_t = nc.dram_tensor("STmask", [P, T * P], FP8, kind="ExternalInput")
    y_t = nc.dram_tensor("y", [npc_pad, nclass], F32, kind="ExternalOutput")
    Hp = nc.dram_tensor("Hp", [HROWS, d], BF16, kind="Internal")

    with tile.TileContext(nc) as tc:
        with (
            tc.tile_pool(name="persist", bufs=1) as pp,
            tc.tile_pool(name="dram", bufs=1, space="DRAM") as dp,
            tc.tile_pool(name="p0", bufs=2) as sb,
            tc.tile_pool(name="mask", bufs=2) as sm,
            tc.tile_pool(name="zpool", bufs=ZBUFS) as sz,
            tc.tile_pool(name="ring", bufs=2) as sr,
            tc.tile_pool(name="epi", bufs=2) as se,
            tc.tile_pool(name="put", bufs=2, space="PSUM") as put,
            tc.tile_pool(name="pseg", bufs=1, space="PSUM") as pse,
            tc.tile_pool(name="ptr", bufs=1, space="PSUM") as ptr,
        ):
            # ---------------- constants / persistent state ----------------
            ident = pp.tile([P, P], BF16)
            make_identity(nc, ident[:])
            ones_sb = pp.tile([1, P], BF16)
            nc.vector.memset(ones_sb[:], 1.0)
            eps_b = pp.tile([P, 1], F32)
            nc.vector.memset(eps_b[:], 1e-24)

            w_sb = pp.tile([P, nkt * d], BF16)
            nc.sync.dma_start(
                out=w_sb[:].rearrange("p (a q) -> p a q", q=d),
                in_=w_t[:].rearrange("(a p) q -> p a q", p=P),
            )
            cw_sb = pp.tile([P, 3 * nclass], BF16)
            nc.sync.dma_start(out=cw_sb[:], in_=cw_t[:])
            src_sb = pp.tile([P, T], I32)
            nc.sync.dma_start(out=src_sb[:], in_=src_t[:])

            hn = pp.tile([P, nchunks * d], BF16)  # normalized features (own nodes)
            ag_in = dp.tile([npc_pad, d], BF16)

            # ---------------- P0: pca + lrelu + l2norm + sub-allgathers ----
            qnext = 0
            for m in range(nchunks):
                xt = sb.tile([P, nkt * P], BF16, tag="xt", bufs=3)
                nc.sync.dma_start(
                    out=xt[:].rearrange("p (a q) -> p a q", q=P),
                    in_=xT_t[:, m * P:(m + 1) * P].rearrange("(a p) q -> p a q", p=P),
                )
                h_ps = put.tile([P, 1024], F32, space="PSUM", tag="ut")
                for a in range(nkt):
                    nc.tensor.matmul(
                        out=h_ps[:, :d],
                        lhsT=xt[:, a * P:(a + 1) * P],
                        rhs=w_sb[:, a * d:(a + 1) * d],
                        start=(a == 0),
                        stop=(a == nkt - 1),
                    )
                hs = sb.tile([P, d], F32, tag="hs")
                nc.vector.tensor_scalar_mul(out=hs[:], in0=h_ps[:, :d], scalar1=SLOPE)
                h = sb.tile([P, d], F32, tag="h")
                nc.vector.tensor_tensor(out=h[:], in0=h_ps[:, :d], in1=hs[:], op=OP.max)
                sq = sb.tile([P, d], F32, tag="sq")
                nc.scalar.activation(out=sq[:], in_=h[:], func=AF.Square)
                ss = sb.tile([P, K], F32, tag="ss")
                nc.vector.reduce_sum(
                    out=ss[:], in_=sq[:].rearrange("p (k e) -> p k e", k=K),
                    axis=AX.X,
                )
                lg = sb.tile([P, K], F32, tag="lg")
                nc.scalar.activation(out=lg[:], in_=ss[:], func=AF.Ln, bias=eps_b[:, :1])
                rr = sb.tile([P, K], F32, tag="rr")
                nc.scalar.activation(out=rr[:], in_=lg[:], func=AF.Exp, scale=-0.5)
                nc.vector.tensor_tensor(
                    out=hn[:, m * d:(m + 1) * d].rearrange("p (k e) -> p k e", k=K),
                    in0=h[:].rearrange("p (k e) -> p k e", k=K),
                    in1=rr[:].unsqueeze(2).to_broadcast([P, K, dd]),
                    op=OP.mult,
                )
                nc.sync.dma_start(
                    out=ag_in[m * P:(m + 1) * P, :], in_=hn[:, m * d:(m + 1) * d]
                )
                if m == bounds[qnext + 1] - 1:
                    q = qnext
                    nc.gpsimd.collective_compute(
                        "AllGather",
                        OP.bypass,
                        replica_groups=[list(range(n_cores))],
                        ins=[ag_in[bounds[q] * P:bounds[q + 1] * P, :]],
                        outs=[Hp.ap()[int(hq_base[q]):int(hq_base[q + 1]), :]],
                    )
                    qnext += 1

            # ---------------- P3: routing ---------------------------------
            def chunk_prologue(j):
                t0, ntj = int(toc[j]), int(nt[j])
                S_sb = sm.tile([P, max_nt * P], FP8, tag=f"S{j % 3}")
                nc.sync.dma_start(
                    out=S_sb[:, :ntj * P], in_=S_t[:, t0 * P:(t0 + ntj) * P]
                )
                ST_sb = sm.tile([P, max_nt * P], FP8, tag=f"ST{j % 3}")
                nc.sync.dma_start(
                    out=ST_sb[:, :ntj * P], in_=ST_t[:, t0 * P:(t0 + ntj) * P]
                )
                zch = sz.tile([P, max_nt * d], BF16, tag="z")
                for b0 in range(0, ntj, ZBATCH):
                    bn = min(ZBATCH, ntj - b0)
                    nc.gpsimd.indirect_dma_start(
                        out=zch[:, b0 * d:(b0 + bn) * d],
                        out_offset=None,
                        in_=Hp.ap(),
                        in_offset=bass.IndirectOffsetOnAxis(
                            ap=src_sb[:, t0 + b0:t0 + b0 + bn], axis=0
                        ),
                    )
                return dict(j=j, ntj=ntj, zch=zch, S_sb=S_sb, ST_sb=ST_sb,
                            u_j=None, plan=_group_plan(ntj))

            def chunk_groups(st, it):
                j, ntj, zch, S_sb, ST_sb = (
                    st["j"], st["ntj"], st["zch"], st["S_sb"], st["ST_sb"])
                hn_j = hn[:, j * d:(j + 1) * d]
                u_rhs = hn_j if it == 0 else st["u_j"][:]
                seg = pse.tile([P, 512], F32, space="PSUM", tag=f"seg{j % 3}")
                st["seg"] = seg
                ti = 0
                for (g0, gn, spans, views) in st["plan"]:
                    utp = put.tile([P, 1024], F32, space="PSUM", tag="ut")
                    for i, t in enumerate(range(g0, g0 + gn)):
                        nc.tensor.matmul(
                            out=utp[:, spans[i]:spans[i] + d],
                            lhsT=ST_sb[:, t * P:(t + 1) * P],
                            rhs=u_rhs,
                            start=True, stop=True,
                        )
                    utb = sr.tile([P, 6 * d], BF16, tag="utb")
                    n0, n1 = views
                    if n1 and n0 == n1:
                        nc.scalar.copy(
                            out=utb[:, :gn * d],
                            in_=utp[:].rearrange("p (b x) -> p b x", b=2)
                                [:, :, :n0 * d].rearrange("p b (i e) -> p b i e", e=d),
                        )
                    else:
                        nc.scalar.copy(out=utb[:, :n0 * d], in_=utp[:, :n0 * d])
                        if n1:
                            nc.scalar.copy(
                                out=utb[:, n0 * d:gn * d],
                                in_=utp[:, 512:512 + n1 * d],
                            )
                    zg = zch[:, g0 * d:(g0 + gn) * d]
                    prod = sr.tile([P, 6 * d], BF16, tag="prod")
                    nc.vector.tensor_mul(out=prod[:, :gn * d], in0=zg, in1=utb[:, :gn * d])
                    pv = prod[:, :gn * d].rearrange("p (a e) -> p a e", e=dd)
                    t1 = sr.tile([P, 6 * d // 2], BF16, tag="t1")
                    nc.vector.tensor_add(
                        out=t1[:, :gn * d // 2].rearrange("p (a e) -> p a e", e=8),
                        in0=pv[:, :, 0:8], in1=pv[:, :, 8:16],
                    )
                    t1v = t1[:, :gn * d // 2].rearrange("p (a e) -> p a e", e=8)
                    t2 = sr.tile([P, 6 * d // 4], BF16, tag="t2")
                    nc.vector.tensor_add(
                        out=t2[:, :gn * d // 4].rearrange("p (a e) -> p a e", e=4),
                        in0=t1v[:, :, 0:4], in1=t1v[:, :, 4:8],
                    )
                    t2v = t2[:, :gn * d // 4].rearrange("p (a e) -> p a e", e=4)
                    t3 = sr.tile([P, 6 * d // 8], BF16, tag="t3")
                    nc.vector.tensor_add(
                        out=t3[:, :gn * d // 8].rearrange("p (a e) -> p a e", e=2),
                        in0=t2v[:, :, 0:2], in1=t2v[:, :, 2:4],
                    )
                    t3v = t3[:, :gn * d // 8].rearrange("p (a e) -> p a e", e=2)
                    sf = sr.tile([P, 6 * K], F32, tag="sf")
                    nc.vector.tensor_add(
                        out=sf[:, :gn * K],
                        in0=t3v[:, :, 0:1].squeeze(2), in1=t3v[:, :, 1:2].squeeze(2),
                    )
                    ef = sr.tile([P, 6 * K], BF16, tag="ef")
                    nc.scalar.activation(out=ef[:, :gn * K], in_=sf[:, :gn * K], func=AF.Exp)
                    qf = sr.tile([P, 6], F32, tag="qf")
                    nc.vector.reduce_sum(
                        out=qf[:, :gn],
                        in_=ef[:, :gn * K].rearrange("p (a k) -> p a k", k=K),
                        axis=AX.X,
                    )
                    rf = sr.tile([P, 6], F32, tag="rf")
                    nc.vector.reciprocal(out=rf[:, :gn], in_=qf[:, :gn])
                    pe2 = sr.tile([P, 6 * K * 2], BF16, tag="pe2")
                    p2v = pe2[:].rearrange("p (a k two) -> p a k two", k=K, two=2)
                    efv = ef[:, :gn * K].rearrange("p (a k) -> p a k", k=K)
                    rfv = rf[:, :gn].unsqueeze(2).to_broadcast([P, gn, K])
                    nc.vector.tensor_tensor(
                        out=p2v[:, :gn, :, 0:1].squeeze(3), in0=efv, in1=rfv, op=OP.mult)
                    nc.vector.tensor_tensor(
                        out=p2v[:, :gn, :, 1:2].squeeze(3), in0=efv, in1=rfv, op=OP.mult)
                    pex = sr.tile([P, 6 * d], BF16, tag="pex")
                    nc.scalar.copy(
                        out=pex.bitcast(F32)[:, :gn * d // 2].rearrange(
                            "p (a e) -> p a e", e=dd // 2),
                        in_=pe2.bitcast(F32)[:, :gn * K].unsqueeze(2).to_broadcast(
                            [P, gn * K, dd // 2]),
                    )
                    msg = sr.tile([P, 6 * d], BF16, tag="msg")
                    nc.vector.tensor_mul(out=msg[:, :gn * d], in0=zg, in1=pex[:, :gn * d])
                    for i, t in enumerate(range(g0, g0 + gn)):
                        nc.tensor.matmul(
                            out=seg[:, :d],
                            lhsT=S_sb[:, t * P:(t + 1) * P],
                            rhs=msg[:, i * d:(i + 1) * d],
                            start=(ti == 0), stop=False,
                        )
                        ti += 1
                # + x residual via identity matmul
                nc.tensor.matmul(
                    out=seg[:, :d], lhsT=ident[:], rhs=hn_j, start=False, stop=True)

            def chunk_epilogue(st, it):
                j, seg = st["j"], st["seg"]
                sq2 = se.tile([P, d], F32, tag="sq2")
                nc.scalar.activation(out=sq2[:], in_=seg[:, :d], func=AF.Square)
                ss2 = se.tile([P, K], F32, tag="ss2")
                nc.vector.reduce_sum(
                    out=ss2[:], in_=sq2[:].rearrange("p (k e) -> p k e", k=K),
                    axis=AX.X,
                )
                lg2 = se.tile([P, K], F32, tag="lg2")
                nc.scalar.activation(out=lg2[:], in_=ss2[:], func=AF.Ln, bias=eps_b[:, :1])
                rr2 = se.tile([P, K], F32, tag="rr2")
                nc.scalar.activation(out=rr2[:], in_=lg2[:], func=AF.Exp, scale=-0.5)
                rrb = rr2[:].unsqueeze(2).to_broadcast([P, K, dd])
                segv = seg[:, :d].rearrange("p (k e) -> p k e", k=K)
                if it < NITER - 1:
                    u_new = se.tile([P, d], BF16, tag=f"uj{j % 3}", bufs=1)
                    nc.vector.tensor_tensor(
                        out=u_new[:].rearrange("p (k e) -> p k e", k=K),
                        in0=segv, in1=rrb, op=OP.mult)
                    st["u_j"] = u_new
                else:
                    uf = se.tile([P, d], F32, tag="uf")
                    nc.vector.tensor_tensor(
                        out=uf[:].rearrange("p (k e) -> p k e", k=K),
                        in0=segv, in1=rrb, op=OP.mult)
                    us = se.tile([P, d], F32, tag="us")
                    nc.vector.tensor_scalar_mul(out=us[:], in0=uf[:], scalar1=SLOPE)
                    ufb = se.tile([P, d], BF16, tag="ufb")
                    nc.vector.tensor_tensor(out=ufb[:], in0=uf[:], in1=us[:], op=OP.max)
                    trp = ptr.tile([P, 1024], BF16, space="PSUM", tag="tr")
                    nc.tensor.transpose(out=trp[:, :P], in_=ufb[:, :P], identity=ident[:])
                    nc.tensor.transpose(
                        out=trp[:d - P, P:2 * P], in_=ufb[:, P:d], identity=ident[:])
                    uT = se.tile([P, 2 * P], BF16, tag="uT")
                    nc.scalar.copy(out=uT[:, :P], in_=trp[:, :P])
                    nc.scalar.copy(out=uT[:d - P, P:], in_=trp[:d - P, P:2 * P])
                    yp = ptr.tile([P, 1024], BF16, space="PSUM", tag="tr")
                    ypv = yp.bitcast(F32)[:, :nclass]
                    nc.tensor.matmul(
                        out=ypv, lhsT=uT[:, :P], rhs=cw_sb[:, :nclass],
                        start=True, stop=False)
                    nc.tensor.matmul(
                        out=ypv, lhsT=uT[:d - P, P:2 * P],
                        rhs=cw_sb[:d - P, nclass:2 * nclass],
                        start=False, stop=False)
                    nc.tensor.matmul(
                        out=ypv, lhsT=ones_sb[:, :P], rhs=cw_sb[0:1, 2 * nclass:],
                        start=False, stop=True)
                    ysb = se.tile([P, nclass], F32, tag="ysb")
                    nc.scalar.copy(out=ysb[:], in_=ypv)
                    nc.sync.dma_start(out=y_t[j * P:(j + 1) * P, :], in_=ysb[:])

            for j0 in range(0, nchunks, 3):
                sts = [chunk_prologue(j) for j in range(j0, min(j0 + 3, nchunks))]
                for it in range(NITER):
                    for st in sts:
                        chunk_groups(st, it)
                    for st in sts:
                        chunk_epilogue(st, it)
    return nc


_CACHE = {}
TRACE = False
LAST_RESULTS = None


def kernel(x, edge_index, pca_w, pca_b, clf_w, clf_b, n_cores=8, _sim=False):
    x = np.asarray(x, np.float32)
    edge_index = np.asarray(edge_index)
    pca_w = np.asarray(pca_w, np.float32)
    pca_b = np.asarray(pca_b, np.float32)
    clf_w = np.asarray(clf_w, np.float32)
    clf_b = np.asarray(clf_b, np.float32)

    n, nfeat = x.shape
    d = pca_w.shape[1]
    nclass = clf_w.shape[1]

    meta, src_dev, S_dev, ST_dev, xT = _host_prep(x, edge_index, n_cores)

    key = (n, nfeat, d, nclass, tuple(meta["nt"].tolist()))
    if key not in _CACHE:
        _CACHE[key] = build_program(nfeat, d, nclass, meta, n_cores)
        if not _sim:
            _split_multiwaits(_CACHE[key])
    nc = _CACHE[key]

    kf_pad = meta["kf_pad"]
    w_pad = np.zeros((kf_pad, d), ml_dtypes.bfloat16)
    w_pad[:nfeat] = pca_w.astype(ml_dtypes.bfloat16)
    w_pad[nfeat] = pca_b.astype(ml_dtypes.bfloat16)
    cwp = np.zeros((P, 3 * nclass), ml_dtypes.bfloat16)
    cwp[:, :nclass] = clf_w[:P].astype(ml_dtypes.bfloat16)
    cwp[:d - P, nclass:2 * nclass] = clf_w[P:].astype(ml_dtypes.bfloat16)
    cwp[0, 2 * nclass:] = clf_b.astype(ml_dtypes.bfloat16)

    in_maps = []
    for c in range(n_cores):
        in_maps.append({
            "xT": xT[c],
            "wp": w_pad,
            "cwp": cwp,
            "src": src_dev[c],
            "Smask": S_dev[c],
            "STmask": ST_dev[c],
        })

    npc = meta["npc"]
    npc_pad = meta["npc_pad"]
    if _sim:
        from concourse.bass_interp import CoreSim
        assert n_cores == 1
        sim = CoreSim(nc)
        for kk, vv in in_maps[0].items():
            sim.tensor(kk)[:] = vv
        sim.simulate()
        y_dev = np.asarray(sim.tensor("y"))[None]
    else:
        global LAST_RESULTS
        res = run_bass_kernel_spmd(
            nc, in_maps, core_ids=list(range(n_cores)), trace=TRACE
        )
        LAST_RESULTS = res
        y_dev = np.stack([res.results[c]["y"] for c in range(n_cores)], axis=0)

    # un-permute: node nd lives at (core, pos)
    y = np.empty((n, nclass), np.float32)
    y[np.arange(n)] = y_dev[meta["node_core"], meta["pos_in_core"]]
    return y.astype(np.float32)


if __name__ == "__main__":
    import pickle, time
    with open("/tmp/ref_inputs.pkl", "rb") as f:
        inputs = pickle.load(f)
    t0 = time.time()
    y = kernel(**inputs)
    print("kernel() wall time", time.time() - t0)
    np.save("/tmp/kernel_out.npy", y)
